# revision 6
# baseline (speedup 1.0000x reference)
"""Episodic LSTM (5-gate) Trainium2 Bass kernel.

Strategy
--------
Data-parallel over batch: B=64 -> 8 cores x 8 batch rows. Weights are
replicated. Everything on-chip lives transposed (feature dim on the 128
SBUF partitions, batch in the free dim) so element-wise work uses all
partitions.

Per core:
  Phase A: x_proj^T = W_ih @ x^T + b as a stationary-weight GEMM over
    T-chunks (bf16 operands, fp32 PSUM), bounced to HBM in bf16.
  Phase B: the T=1024 sequential recurrence. Per step, per chain:
    - one identity-stationary matmul injects x_proj_t into PSUM
      (start=True), then 20 (LDWEIGHTS+MATMUL) pairs accumulate
      W_hh^T tiles against h^T (bf16, N=batch). Gates come out in
      gate-major layout [128, 10, b].
    - ScalarE applies sigmoid to all 5 gates in one op. tanh is
      computed via tanh(z) = 2*sigmoid(2z) - 1; the g-gate rows of
      W_ih/W_hh/b are pre-scaled by 2 on the host so no extra scaling
      op is needed.
    - VectorE computes the c/h updates in fp32 (c state stays fp32).
    - tanh(c) again via sigmoid(2c) using the activation scale field.
  Two independent chains (batch 4+4) are interleaved to hide the
  serial per-step latency (ACT/DVE fixed costs + sync) under the other
  chain's matmul phase.

Gate order is host-reordered to [i, f, g, r, o].
"""

import os
import sys

import numpy as np
import ml_dtypes

import concourse.bass as bass
import concourse.bacc as bacc
import concourse.tile as tile
from concourse import mybir
from concourse.bass_utils import run_bass_kernel_spmd

BF16 = mybir.dt.bfloat16
F32 = mybir.dt.float32

T_LEN = 1024
BATCH = 64
IN_DIM = 256
HID = 256
N_CORES = 8
BPC = BATCH // N_CORES          # batch per core = 8
N_CHAINS = 2
BCH = BPC // N_CHAINS           # batch per chain = 4
KC = 2                          # 256 = 2 x 128 contraction chunks
GC = 10                         # 1280 = 10 x 128 gate chunks
TC = 128                        # T chunk
N_CHUNKS = T_LEN // TC

_PROG_CACHE = {}
_last_in_maps = None


def _build_program(t_len=T_LEN, tc=TC, n_chains=N_CHAINS, seq_f32=True):
    n_chunks = t_len // tc
    bch = BPC // n_chains
    nc = bacc.Bacc("TRN2", target_bir_lowering=False, debug=False,
                   num_devices=N_CORES)

    seq_dt = F32 if seq_f32 else BF16

    # ---- DRAM I/O ----
    xT = nc.dram_tensor("xT", [128, KC, t_len, BPC], BF16,
                        kind="ExternalInput").ap()
    mT = nc.dram_tensor("mT", [128, KC, t_len, BPC], F32,
                        kind="ExternalInput").ap()
    wih = nc.dram_tensor("wih", [128, KC, GC, 128], BF16,
                         kind="ExternalInput").ap()
    whh = nc.dram_tensor("whh", [128, KC, GC, 128], BF16,
                         kind="ExternalInput").ap()
    bT = nc.dram_tensor("bT", [128, GC], F32, kind="ExternalInput").ap()
    h0T = nc.dram_tensor("h0T", [128, KC, BPC], BF16,
                         kind="ExternalInput").ap()
    c0T = nc.dram_tensor("c0T", [128, KC, BPC], F32,
                         kind="ExternalInput").ap()
    ident = nc.dram_tensor("ident", [128, 128], BF16,
                           kind="ExternalInput").ap()

    seqT = nc.dram_tensor("seqT", [128, KC, t_len, BPC], seq_dt,
                          kind="ExternalOutput").ap()
    cT_out = nc.dram_tensor("cT_out", [128, KC, BPC], F32,
                            kind="ExternalOutput").ap()

    # internal HBM bounce for x_proj^T (bf16)
    xproj_dram = nc.dram_tensor("xproj_bounce", [128, GC, t_len, BPC],
                                BF16).ap()

    with tile.TileContext(nc) as tc_ctx:
        _emit(nc, tc_ctx, locals(), t_len, tc, n_chunks, n_chains, bch,
              seq_dt)

    nc.compile()
    return nc


def _emit(nc, tc_ctx, aps, t_len, tcw, n_chunks, n_chains, bch, seq_dt):
    xT, mT, wih, whh, bT = aps["xT"], aps["mT"], aps["wih"], aps["whh"], aps["bT"]
    h0T, c0T, ident = aps["h0T"], aps["c0T"], aps["ident"]
    seqT, cT_out, xproj_dram = aps["seqT"], aps["cT_out"], aps["xproj_dram"]

    Sig = mybir.ActivationFunctionType.Sigmoid
    Ident = mybir.ActivationFunctionType.Identity
    MUL = mybir.AluOpType.mult
    ADD = mybir.AluOpType.add

    with (
        tc_ctx.tile_pool(name="const", bufs=1) as cpool,
        tc_ctx.tile_pool(name="xa", bufs=2) as xa_pool,
        tc_ctx.tile_pool(name="xpa", bufs=2) as xpa_pool,
        tc_ctx.tile_pool(name="psA", bufs=2, space="PSUM") as psA_pool,
        tc_ctx.tile_pool(name="xb", bufs=2) as xb_pool,
        tc_ctx.tile_pool(name="mb", bufs=2) as mb_pool,
        tc_ctx.tile_pool(name="sq", bufs=2) as sq_pool,
        tc_ctx.tile_pool(name="psB", bufs=2, space="PSUM") as psB_pool,
        tc_ctx.tile_pool(name="gt", bufs=4) as g_pool,
        tc_ctx.tile_pool(name="st", bufs=4) as st_pool,
        tc_ctx.tile_pool(name="hc", bufs=4) as hc_pool,
    ):
        # ---- persistent tiles ----
        wih_sb = cpool.tile([128, KC, GC, 128], BF16)
        nc.sync.dma_start(wih_sb[:], wih[:])
        whh_sb = cpool.tile([128, KC, GC, 128], BF16)
        nc.sync.dma_start(whh_sb[:], whh[:])
        b_sb = cpool.tile([128, GC], F32)
        nc.sync.dma_start(b_sb[:], bT[:])
        id_sb = cpool.tile([128, 128], BF16)
        nc.sync.dma_start(id_sb[:], ident[:])

        # ---- Phase A: x_proj GEMM, chunk by chunk, bounced to HBM ----
        th = tcw // 2  # time-steps per PSUM-limited matmul (<=512 cols)
        for ch in range(n_chunks):
            xa = xa_pool.tile([128, KC, tcw, BPC], BF16, tag="xa")
            nc.sync.dma_start(xa[:], xT[:, :, ch * tcw:(ch + 1) * tcw, :])
            xpa = xpa_pool.tile([128, GC, tcw, BPC], BF16, tag="xpa")
            for m in range(GC):
                for hf in range(2):
                    ps = psA_pool.tile([128, th, BPC], F32, tag="psA")
                    for k in range(KC):
                        nc.tensor.matmul(
                            ps[:],
                            wih_sb[:, k, m, :],
                            xa[:, k, hf * th:(hf + 1) * th, :],
                            start=(k == 0), stop=(k == KC - 1),
                        )
                    dst = xpa[:, m, hf * th:(hf + 1) * th, :]
                    if hf == 0:
                        nc.vector.tensor_scalar(
                            dst, ps[:], b_sb[:, m:m + 1], None, ADD)
                    else:
                        nc.scalar.activation(dst, ps[:], Ident,
                                             bias=b_sb[:, m:m + 1])
            nc.sync.dma_start(
                xproj_dram[:, :, ch * tcw:(ch + 1) * tcw, :], xpa[:])

        # ---- Phase B: recurrence ----
        # chain state
        h_cur = []
        c_cur = []
        for cn in range(n_chains):
            ht = hc_pool.tile([128, KC, bch], BF16, tag=f"h{cn}")
            nc.sync.dma_start(ht[:], h0T[:, :, cn * bch:(cn + 1) * bch])
            ct = hc_pool.tile([128, KC, bch], F32, tag=f"c{cn}")
            nc.sync.dma_start(ct[:], c0T[:, :, cn * bch:(cn + 1) * bch])
            h_cur.append(ht)
            c_cur.append(ct)

        for ch in range(n_chunks):
            xp = xb_pool.tile([128, GC, tcw, BPC], BF16, tag="xp")
            nc.sync.dma_start(xp[:], xproj_dram[:, :, ch * tcw:(ch + 1) * tcw, :])
            mb = mb_pool.tile([128, KC, tcw, BPC], F32, tag="mb")
            nc.sync.dma_start(mb[:], mT[:, :, ch * tcw:(ch + 1) * tcw, :])
            sq = sq_pool.tile([128, KC, tcw, BPC], seq_dt, tag="sq")

            for ti in range(tcw):
                for cn in range(n_chains):
                    b0, b1 = cn * bch, (cn + 1) * bch
                    ps = psB_pool.tile([128, GC, bch], F32, tag=f"ps{cn}")
                    # inject x_proj_t (identity stationary)
                    nc.tensor.matmul(
                        ps[:, :, :], id_sb[:, :], xp[:, :, ti, b0:b1],
                        start=True, stop=False, skip_group_check=True)
                    for m in range(GC):
                        for k in range(KC):
                            nc.tensor.matmul(
                                ps[:, m, :], whh_sb[:, k, m, :],
                                h_cur[cn][:, k, :],
                                start=False,
                                stop=(m == GC - 1 and k == KC - 1),
                                skip_group_check=True)
                    # gates: sigmoid everything (g-rows pre-scaled x2)
                    S = st_pool.tile([128, GC, bch], F32, tag=f"S{cn}")
                    nc.scalar.activation(S[:], ps[:], Sig)
                    Si, Sf = S[:, 0:KC, :], S[:, KC:2 * KC, :]
                    Sg2, Sr = S[:, 2 * KC:3 * KC, :], S[:, 3 * KC:4 * KC, :]
                    So = S[:, 4 * KC:5 * KC, :]
                    # c' = Sf*c + Si*(2*Sg2-1) + Sr*m_t
                    u = g_pool.tile([128, KC, bch], F32, tag=f"u{cn}")
                    nc.vector.tensor_scalar(u[:], Sg2, 2.0, -1.0, MUL, ADD)
                    p = g_pool.tile([128, KC, bch], F32, tag=f"p{cn}")
                    nc.vector.tensor_mul(p[:], Si, u[:])
                    q = g_pool.tile([128, KC, bch], F32, tag=f"q{cn}")
                    nc.vector.tensor_mul(q[:], Sf, c_cur[cn][:])
                    r2 = g_pool.tile([128, KC, bch], F32, tag=f"r{cn}")
                    nc.vector.tensor_mul(
                        r2[:], Sr, mb[:, :, ti, b0:b1])
                    s1 = g_pool.tile([128, KC, bch], F32, tag=f"s{cn}")
                    nc.vector.tensor_add(s1[:], p[:], q[:])
                    c_new = hc_pool.tile([128, KC, bch], F32, tag=f"c{cn}")
                    nc.vector.tensor_add(c_new[:], s1[:], r2[:])
                    # h = So * (2*sigmoid(2c)-1)
                    S2c = g_pool.tile([128, KC, bch], F32, tag=f"t{cn}")
                    nc.scalar.activation(S2c[:], c_new[:], Sig, scale=2.0)
                    v = g_pool.tile([128, KC, bch], F32, tag=f"v{cn}")
                    nc.vector.tensor_scalar(v[:], S2c[:], 2.0, -1.0, MUL, ADD)
                    nc.vector.tensor_mul(
                        sq[:, :, ti, b0:b1], So, v[:])
                    h_new = hc_pool.tile([128, KC, bch], BF16, tag=f"h{cn}")
                    nc.vector.tensor_mul(h_new[:], So, v[:])
                    h_cur[cn] = h_new
                    c_cur[cn] = c_new
            nc.sync.dma_start(
                seqT[:, :, ch * tcw:(ch + 1) * tcw, :], sq[:])

        for cn in range(n_chains):
            nc.sync.dma_start(
                cT_out[:, :, cn * bch:(cn + 1) * bch], c_cur[cn][:])


# ---------------- host-side packing ----------------

def _reorder_rows(w):
    """[i,f,g,o,r] row blocks -> [i,f,g,r,o], g-rows scaled by 2."""
    H = HID
    i, f, g, o, r = (w[j * H:(j + 1) * H] for j in range(5))
    return np.concatenate([i, f, 2.0 * g, r, o], axis=0)


def _pack_w(w):  # [1280, 256] -> [128, KC, GC, 128]
    wn = _reorder_rows(np.asarray(w, np.float32))
    wt = wn.T.reshape(KC, 128, GC, 128).transpose(1, 0, 2, 3)
    return np.ascontiguousarray(wt).astype(ml_dtypes.bfloat16)


def _pack_seq_in(x, dtype):  # [T, b, 256] -> [128, KC, T, b]
    t, b, d = x.shape
    xt = np.asarray(x, np.float32).transpose(2, 0, 1).reshape(KC, 128, t, b)
    return np.ascontiguousarray(xt.transpose(1, 0, 2, 3)).astype(dtype)


def _pack_state(s, dtype):  # [b, 256] -> [128, KC, b]
    st = np.asarray(s, np.float32).T.reshape(KC, 128, -1)
    return np.ascontiguousarray(st.transpose(1, 0, 2)).astype(dtype)


def kernel(x, m, h0, c0, W_ih, W_hh, b, _t_len=None):
    t_len = _t_len or x.shape[0]
    key = (t_len,)
    if key not in _PROG_CACHE:
        tcw = min(TC, t_len)
        _PROG_CACHE[key] = _build_program(t_len=t_len, tc=tcw)
    nc = _PROG_CACHE[key]

    wih_p = _pack_w(W_ih)
    whh_p = _pack_w(W_hh)
    bn = _reorder_rows(np.asarray(b, np.float32).reshape(5 * HID, 1))[:, 0]
    b_p = np.ascontiguousarray(bn.reshape(GC, 128).T).astype(np.float32)
    id_p = np.eye(128, dtype=ml_dtypes.bfloat16)

    in_maps = []
    for c in range(N_CORES):
        bs = slice(c * BPC, (c + 1) * BPC)
        in_maps.append({
            "xT": _pack_seq_in(x[:, bs, :], ml_dtypes.bfloat16),
            "mT": _pack_seq_in(m[:, bs, :], np.float32),
            "wih": wih_p, "whh": whh_p, "bT": b_p,
            "h0T": _pack_state(h0[bs], ml_dtypes.bfloat16),
            "c0T": _pack_state(c0[bs], np.float32),
            "ident": id_p,
        })

    global _last_in_maps
    _last_in_maps = in_maps
    res = run_bass_kernel_spmd(nc, in_maps, list(range(N_CORES)))

    seqs, cts = [], []
    for c in range(N_CORES):
        sq = np.asarray(res.results[c]["seqT"], np.float32)
        # [128, KC, T, b] -> [T, b, 256]
        seqs.append(sq.transpose(2, 3, 1, 0).reshape(t_len, BPC, HID))
        ct = np.asarray(res.results[c]["cT_out"], np.float32)
        cts.append(ct.transpose(2, 1, 0).reshape(BPC, HID))
    seq = np.concatenate(seqs, axis=1)
    cT = np.concatenate(cts, axis=0)
    hT = seq[-1].copy()
    return seq, hT, cT
